# revision 1
# baseline (speedup 1.0000x reference)
"""LIF (leaky integrate-and-fire) spiking neuron kernel for Trainium2.

Reference semantics (T=4 timesteps, f32):
    mem = 0
    for t: mem = mem + x_t; spike_t = (mem >= 1.0); mem = (1 - spike_t) * mem
Output: spikes, same shape/dtype as input x [T*B, C, H, W] = [128,128,56,56] f32.

Strategy: pure data-parallel over batch. Each of 8 cores gets B_loc=4 of the
B=32 batch entries: a [T=4, N=1605632] f32 slab. On-core the N axis is tiled
as [NCH=8, P=128, F=1568]. Per chunk: load the 4 timestep tiles (SP HWDGE
ring), run the exact f32 LIF recurrence on the Vector engine, emit spikes as
bf16 (0/1 exact in bf16) on the ACT HWDGE ring to halve store traffic, upcast
to f32 on host.

Raw Block-based bass with standalone wait_ge instructions (this container's
walrus rejects >1 attached sync-wait on TT/STT instructions, which rules out
Tile-generated schedules). Double-buffered: loads of chunk c overlap compute
of chunk c-1 and stores of chunk c-2. All compute is IEEE f32 matching the
reference op-for-op, so the result is bit-exact.
"""

import sys

for _p in ("/opt/trn_rl_repo",):
    if _p not in sys.path:
        sys.path.insert(0, _p)

import numpy as np

T = 4
B = 32
C, H, W = 128, 56, 56
CHW = C * H * W          # 401408
M = 8                    # cores
B_LOC = B // M           # 4
N = B_LOC * CHW          # 1605632 elements per timestep per core
P = 128
F = 1568
NCH = N // (P * F)       # 8 chunks
NBUF = 3                 # buffer depth (x and spike tiles)
assert NCH * P * F == N

NV = 10                  # DVE ops per chunk (2 + 3 + 3 + 2)
# v_sem count right after spike t of a chunk is written (1-indexed offsets)
V_OFF = {0: 1, 1: 4, 2: 7, 3: 10}

_NC_CACHE = None


def _build():
    from contextlib import ExitStack

    import concourse.bass as bass
    import concourse.mybir as mybir

    fp32 = mybir.dt.float32
    bf16 = mybir.dt.bfloat16
    Alu = mybir.AluOpType

    nc = bass.Bass()
    x = nc.dram_tensor("x", [T, NCH, P, F], fp32, kind="ExternalInput")
    y = nc.dram_tensor("y", [T, NCH, P, F], bf16, kind="ExternalOutput")

    with ExitStack() as ctx:
        xb = [[ctx.enter_context(nc.sbuf_tensor(f"xb{t}_{k}", [P, F], fp32))
               for k in range(NBUF)] for t in range(T)]
        sb = [[ctx.enter_context(nc.sbuf_tensor(f"sb{t}_{k}", [P, F], bf16))
               for k in range(NBUF)] for t in range(T)]
        mm = ctx.enter_context(nc.sbuf_tensor("mm", [P, F], fp32))
        mr = ctx.enter_context(nc.sbuf_tensor("mr", [P, F], fp32))
        # One load/store semaphore per timestep: DMA completions across the
        # 16 SDMA engines are NOT ordered, so a single counting semaphore
        # can't tell WHICH transfer finished. Per-t sems make each wait
        # target the exact transfer (per t, transfers are serial: one per
        # chunk).
        in_sems = [ctx.enter_context(nc.semaphore(f"in_sem{t}")) for t in range(T)]
        out_sems = [ctx.enter_context(nc.semaphore(f"out_sem{t}")) for t in range(T)]
        v_sem = ctx.enter_context(nc.semaphore("v_sem"))
        block = ctx.enter_context(nc.Block())

        @block.sync
        def _(sync):
            for c in range(NCH):
                k = c % NBUF
                if c >= NBUF:
                    # chunk c-NBUF's DVE work fully done -> x buffers free
                    sync.wait_ge(v_sem, NV * (c - NBUF + 1))
                for t in range(T):
                    sync.dma_start(out=xb[t][k][:], in_=x[t, c]).then_inc(
                        in_sems[t], 16
                    )

        @block.vector
        def _(vector):
            for c in range(NCH):
                k = c % NBUF
                if c >= NBUF:
                    # chunk c-NBUF's spike stores for each t done -> s free
                    for t in range(T):
                        vector.wait_ge(out_sems[t], 16 * (c - NBUF + 1))
                m = xb[0][k]
                for t in range(T):
                    vector.wait_ge(in_sems[t], 16 * (c + 1))
                    if t > 0:
                        nc.vector.tensor_tensor(
                            out=mm[:], in0=mr[:], in1=xb[t][k][:], op=Alu.add
                        ).then_inc(v_sem, 1)
                        m = mm
                    # spike_t = (m >= 1.0), exact 0.0/1.0, cast to bf16
                    nc.vector.tensor_scalar(
                        out=sb[t][k][:], in0=m[:], scalar1=1.0, scalar2=None,
                        op0=Alu.is_ge,
                    ).then_inc(v_sem, 1)
                    if t < T - 1:
                        # hard reset: mr = (m < 1.0) * m (one fused DVE op)
                        nc.vector.scalar_tensor_tensor(
                            out=mr[:], in0=m[:], scalar=1.0, in1=m[:],
                            op0=Alu.is_lt, op1=Alu.mult,
                        ).then_inc(v_sem, 1)

        @block.scalar
        def _(scalar):
            for c in range(NCH):
                k = c % NBUF
                for t in range(T):
                    scalar.wait_ge(v_sem, NV * c + V_OFF[t])
                    scalar.dma_start(out=y[t, c], in_=sb[t][k][:]).then_inc(
                        out_sems[t], 16
                    )

    return nc


def _get_nc():
    global _NC_CACHE
    if _NC_CACHE is None:
        _NC_CACHE = _build()
    return _NC_CACHE


def run(x, trace=False, **kwargs):
    """Returns (full f32 spike output, BassKernelResults)."""
    from concourse.bass_utils import run_bass_kernel_spmd

    x = np.asarray(x)
    assert x.shape == (T * B, C, H, W) and x.dtype == np.float32

    # [T*B, C, H, W] -> [T, B, CHW]; shard batch across cores (views only)
    xb = x.reshape(T, B, CHW)
    in_maps = [
        {"x": xb[:, m * B_LOC:(m + 1) * B_LOC].reshape(T, NCH, P, F)}
        for m in range(M)
    ]

    res = run_bass_kernel_spmd(
        _get_nc(), in_maps, core_ids=list(range(M)), trace=trace, **kwargs
    )

    out = np.empty((T, B, CHW), dtype=np.float32)
    for m in range(M):
        out[:, m * B_LOC:(m + 1) * B_LOC] = (
            np.asarray(res.results[m]["y"])
            .astype(np.float32)
            .reshape(T, B_LOC, CHW)
        )
    return out.reshape(T * B, C, H, W), res


def kernel(x):
    return run(x)[0]

